# revision 14
# baseline (speedup 1.0000x reference)
"""Additive-attention kernel for Trainium2, data-parallel over 8 NeuronCores.

Reference computation (B=16, S=2048, H=1024):
    concat = [broadcast(q), keys]                 # [B,S,2H]
    h      = relu(concat @ W1 + b1)               # [B,S,H]
    scores = h @ W2 + b2                          # [B,S,1]
    alphas = softmax(scores, axis=S)
    context = alphas^T @ values                   # [B,1,H]
    returns (context, alphas)

Kernel strategy (per core, 2 batches):
  - concat @ W1 == q @ W1[:H] + keys @ W1[H:]  -> the q part is computed once
    per batch ([B,H]) in fp32r, folded with b1 into a per-partition bias; the
    big matmul is keys @ W1k only (half the naive FLOPs).
  - The keys/values datapath runs in bf16 (cast in the DMA datapath by
    gpsimd DMAs): hT[h',s] = relu(W1k^T @ keys^T + qpre) with keys tiles
    PE-transposed on chip.
  - scores come out as rows [1,512]; exp on ACT with accum_out giving the
    partial softmax denominators for free. b2 is dropped entirely (softmax is
    shift-invariant) and so is the max-subtraction (scores are provably small
    for this module, so exp cannot overflow in fp32).
  - context = (sum_s exp(s) * values[s]) * (1/Z), accumulated in PSUM (fp32)
    across the whole sequence, normalized once at the end.
"""

import sys

for _p in ("/opt/trn_rl_repo",):
    if _p not in sys.path:
        sys.path.append(_p)

import numpy as np

B, S, H = 16, 2048, 1024
NCORES = 8
BPC = B // NCORES          # batches per core
P = 128                    # partitions
FO = H // P                # 8 f-chunks (contraction dim of W1k)
MO = H // P                # 8 h'-chunks
SBLK = 512                 # s-block (matmul moving free dim)
NST = S // SBLK            # 4 s-blocks per batch
NSO = SBLK // P            # 4 s-subchunks per block

_NC_CACHE = {}


def _build_nc():
    import concourse.bass as bass  # noqa: F401
    import concourse.mybir as mybir
    import concourse.tile as tile
    from concourse import bacc
    from concourse.bass import ts, ds
    from concourse.masks import make_identity

    dt = mybir.dt
    f32, f32r, bf16 = dt.float32, dt.float32r, dt.bfloat16
    AF = mybir.ActivationFunctionType

    nc = bacc.Bacc()

    keys_d = nc.dram_tensor("keys", [BPC, S, H], f32, kind="ExternalInput")
    values_d = nc.dram_tensor("values", [BPC, S, H], f32, kind="ExternalInput")
    # host-prepped layouts: [P, FO, ...] with the contraction dim on partitions
    w1k_d = nc.dram_tensor("w1k", [P, FO, H], f32, kind="ExternalInput")
    w1q_d = nc.dram_tensor("w1q", [P, FO, H], f32, kind="ExternalInput")
    qT_d = nc.dram_tensor("qT", [P, FO, BPC], f32, kind="ExternalInput")
    b1T_d = nc.dram_tensor("b1T", [P, MO], f32, kind="ExternalInput")
    w2T_d = nc.dram_tensor("w2T", [P, MO], f32, kind="ExternalInput")

    ctx_d = nc.dram_tensor("context", [BPC, H], f32, kind="ExternalOutput")
    alphas_d = nc.dram_tensor("alphas", [BPC, S], f32, kind="ExternalOutput")

    with tile.TileContext(nc) as tc:
        with (
            tc.tile_pool(name="const", bufs=1) as const_pool,
            tc.tile_pool(name="w1qp", bufs=2) as w1q_pool,
            tc.tile_pool(name="knat", bufs=3) as knat_pool,
            tc.tile_pool(name="vnat", bufs=4) as vnat_pool,
            tc.tile_pool(name="ktsb", bufs=2) as kt_pool,
            tc.tile_pool(name="htsb", bufs=2) as ht_pool,
            tc.tile_pool(name="alph", bufs=2) as alph_pool,
            tc.tile_pool(name="small", bufs=2) as small_pool,
            tc.tile_pool(name="ktps", bufs=2, space="PSUM") as ktp_pool,
            tc.tile_pool(name="hps", bufs=2, space="PSUM") as h_psum_pool,
            tc.tile_pool(name="scps", bufs=1, space="PSUM") as sc_pool,
            tc.tile_pool(name="etps", bufs=1, space="PSUM") as etp_pool,
            tc.tile_pool(name="ctxps", bufs=2, space="PSUM") as ctx_pool,
        ):
            # ---- prefetch the first keys/values blocks (gpsimd queue head,
            # casting fp32 -> bf16 in the DMA datapath) ----
            kv_tiles = {}

            def issue_kv(b, st):
                knat = knat_pool.tile(
                    [P, NSO, H], bf16, tag="knat", name=f"knat_{b}_{st}"
                )
                nc.gpsimd.dma_start(
                    knat[:],
                    keys_d[b, ts(st, SBLK), :].rearrange(
                        "(so si) f -> si so f", si=P
                    ),
                )
                vnat = []
                for hb in range(2):
                    vt = vnat_pool.tile(
                        [P, 2, H], bf16, tag="vnat", name=f"vnat_{b}_{st}_{hb}"
                    )
                    nc.gpsimd.dma_start(
                        vt[:],
                        values_d[b, ds(st * SBLK + hb * 2 * P, 2 * P), :].rearrange(
                            "(so si) f -> si so f", si=P
                        ),
                    )
                    vnat.append(vt)
                return knat, vnat

            kv_tiles[(0, 0)] = issue_kv(0, 0)

            # ---- constants / weights ----
            iden = const_pool.tile([P, P], f32)
            make_identity(nc, iden)
            iden_bf = const_pool.tile([P, P], bf16)
            nc.vector.tensor_copy(out=iden_bf[:], in_=iden[:])

            qT_sb = const_pool.tile([P, FO, BPC], f32r)
            nc.gpsimd.dma_start(qT_sb[:], qT_d[:])
            w2T_sb = const_pool.tile([P, MO], bf16)
            nc.gpsimd.dma_start(w2T_sb[:], w2T_d[:])
            b1T_sb = const_pool.tile([P, MO], f32)
            nc.sync.dma_start(b1T_sb[:], b1T_d[:])

            kv_tiles[(0, 1)] = issue_kv(0, 1)

            # W1 rides the sync (HWDGE) queue in fp32 chunks so it does not
            # serialize behind the keys/values stream; DVE rounds the chunks
            # (W1q to f32r for the small query matmul, W1k to bf16).
            w1k_sb = const_pool.tile([P, FO, H], bf16)
            qpreT_sb = const_pool.tile([P, MO, BPC], f32)
            qrow_ps = [
                ctx_pool.tile([BPC, SBLK], f32, tag="ctx", name=f"qrow{n}")
                for n in range(2)
            ]
            for fo in range(FO):
                w1q_raw = w1q_pool.tile([P, H], f32, tag="wraw", name=f"w1qraw{fo}")
                nc.sync.dma_start(w1q_raw[:], w1q_d[:, fo, :])
                w1q_sb = w1q_pool.tile([P, H], f32r, tag="w1q")
                nc.vector.tensor_copy(out=w1q_sb[:], in_=w1q_raw[:])
                # qrow[b, h'] = qT.T @ W1q  (queries are the 2-col stationary)
                for n in range(2):
                    nc.tensor.matmul(
                        qrow_ps[n][:],
                        lhsT=qT_sb[:, fo, :],
                        rhs=w1q_sb[:, ts(n, SBLK)],
                        start=(fo == 0),
                        stop=(fo == FO - 1),
                    )
            for fo in range(FO):
                wraw = w1q_pool.tile([P, H], f32, tag="wraw", name=f"w1kraw{fo}")
                nc.sync.dma_start(wraw[:], w1k_d[:, fo, :])
                nc.vector.tensor_copy(out=w1k_sb[:, fo, :], in_=wraw[:])

            # qpreT[h', b] = qrow^T + b1 broadcast (PE-transposed in P-chunks)
            qrow_sb = const_pool.tile([BPC, H], f32)
            for n in range(2):
                nc.vector.tensor_copy(
                    out=qrow_sb[:, ts(n, SBLK)], in_=qrow_ps[n][:]
                )
            for mo in range(MO):
                qtp = etp_pool.tile([P, NSO], f32, tag="etp", name=f"qtp{mo}")
                nc.tensor.transpose(
                    qtp[:, :BPC],
                    qrow_sb[:, ts(mo, P)],
                    iden[:BPC, :BPC],
                )
                nc.vector.tensor_tensor(
                    qpreT_sb[:, mo, :],
                    qtp[:, :BPC],
                    b1T_sb[:, mo : mo + 1].to_broadcast((P, BPC)),
                    mybir.AluOpType.add,
                )

            # ---- main loop ----
            for b in range(BPC):
                ctx_ps = [
                    ctx_pool.tile([1, SBLK], f32, tag="ctx", name=f"ctx_{b}_{n}")
                    for n in range(2)
                ]
                alph = alph_pool.tile([1, S], f32)
                zpart = small_pool.tile([1, NST], f32, tag="zpart")

                for st in range(NST):
                    knat, vnat = kv_tiles.pop((b, st), None) or issue_kv(b, st)
                    nxt = (b, st + 1) if st + 1 < NST else (b + 1, 0)
                    if nxt[0] < BPC and nxt not in kv_tiles:
                        kv_tiles[nxt] = issue_kv(*nxt)

                    # keys^T tiles: [f-chunk on partitions, s]
                    kT = kt_pool.tile([P, FO, SBLK], bf16)
                    for fo in range(FO):
                        ktp = ktp_pool.tile([P, SBLK], bf16, tag="ktp")
                        for so in range(NSO):
                            nc.tensor.transpose(
                                ktp[:, ts(so, P)],
                                knat[:, so, ts(fo, P)],
                                iden_bf[:],
                            )
                        nc.vector.tensor_copy(out=kT[:, fo, :], in_=ktp[:])

                    # hT[h'-chunk, s] = relu(W1k^T @ keys^T + qpre)
                    hT = ht_pool.tile([P, MO, SBLK], bf16)
                    for mo in range(MO):
                        hps = h_psum_pool.tile([P, SBLK], f32, tag="hps")
                        for fo in range(FO):
                            nc.tensor.matmul(
                                hps,
                                lhsT=w1k_sb[:, fo, ts(mo, P)],
                                rhs=kT[:, fo, :],
                                start=(fo == 0),
                                stop=(fo == FO - 1),
                            )
                        nc.scalar.activation(
                            out=hT[:, mo, :],
                            in_=hps,
                            func=AF.Relu,
                            bias=qpreT_sb[:, mo, b : b + 1],
                            scale=1.0,
                        )

                    # scores row [1, SBLK] = w2^T @ hT
                    scp = sc_pool.tile([1, SBLK], f32, tag="scp")
                    for mo in range(MO):
                        nc.tensor.matmul(
                            scp,
                            lhsT=w2T_sb[:, mo : mo + 1],
                            rhs=hT[:, mo, :],
                            start=(mo == 0),
                            stop=(mo == MO - 1),
                        )

                    # exp + partial softmax denominator in one ACT op
                    nc.scalar.activation(
                        out=alph[:, ts(st, SBLK)],
                        in_=scp,
                        func=AF.Exp,
                        accum_out=zpart[:, st : st + 1],
                    )

                    # transpose exp row -> [s on partitions, 1] for context
                    etp = etp_pool.tile([P, NSO], f32, tag="etp")
                    for so in range(NSO):
                        nc.tensor.transpose(
                            etp[:, so : so + 1],
                            alph[:, ds(st * SBLK + so * P, P)],
                            iden[0:1, 0:1],
                        )
                    expT = small_pool.tile([P, NSO], bf16, tag="expT")
                    nc.vector.tensor_copy(out=expT[:], in_=etp[:])

                    # context accumulation: ctx[n] += expT^T @ values
                    for n in range(2):
                        for so in range(NSO):
                            nc.tensor.matmul(
                                ctx_ps[n],
                                lhsT=expT[:, so : so + 1],
                                rhs=vnat[so // 2][:, so % 2, ts(n, SBLK)],
                                start=(st == 0 and so == 0),
                                stop=(st == NST - 1 and so == NSO - 1),
                            )

                # ---- batch epilogue: normalize ----
                zsum = small_pool.tile([1, 1], f32, tag="zsum")
                nc.vector.reduce_sum(zsum, zpart, axis=mybir.AxisListType.X)
                rinv = small_pool.tile([1, 1], f32, tag="rinv")
                nc.vector.reciprocal(rinv, zsum)

                nc.vector.tensor_scalar_mul(alph[:], alph[:], rinv)
                nc.sync.dma_start(alphas_d[b : b + 1, :], alph[:])

                ctx_sb = small_pool.tile([1, H], f32, tag="ctxsb")
                for n in range(2):
                    nc.vector.tensor_scalar_mul(
                        ctx_sb[:, ts(n, SBLK)], ctx_ps[n][:], rinv
                    )
                nc.sync.dma_start(ctx_d[b : b + 1, :], ctx_sb[:])

    nc.finalize()
    return nc


def _get_nc():
    if "nc" not in _NC_CACHE:
        _NC_CACHE["nc"] = _build_nc()
    return _NC_CACHE["nc"]


def make_in_maps(queries, keys, values, W1, b1, W2):
    w1k_h = np.ascontiguousarray(W1[H:].reshape(FO, P, H).transpose(1, 0, 2))
    w1q_h = np.ascontiguousarray(W1[:H].reshape(FO, P, H).transpose(1, 0, 2))
    b1T_h = np.ascontiguousarray(b1.reshape(MO, P).T)
    w2T_h = np.ascontiguousarray(W2[:, 0].reshape(MO, P).T)

    in_maps = []
    for c in range(NCORES):
        bsl = slice(c * BPC, (c + 1) * BPC)
        qT_h = np.ascontiguousarray(
            queries[bsl].T.reshape(FO, P, BPC).transpose(1, 0, 2)
        )
        in_maps.append(
            {
                "keys": np.ascontiguousarray(keys[bsl]),
                "values": np.ascontiguousarray(values[bsl]),
                "w1k": w1k_h,
                "w1q": w1q_h,
                "qT": qT_h,
                "b1T": b1T_h,
                "w2T": w2T_h,
            }
        )
    return in_maps


def kernel(queries, keys, values, W1, b1, W2, b2, **_ignored):
    from concourse.bass_utils import run_bass_kernel_spmd

    queries = np.asarray(queries, dtype=np.float32)
    keys = np.asarray(keys, dtype=np.float32)
    values = np.asarray(values, dtype=np.float32)
    W1 = np.asarray(W1, dtype=np.float32)
    b1 = np.asarray(b1, dtype=np.float32)
    W2 = np.asarray(W2, dtype=np.float32)

    in_maps = make_in_maps(queries, keys, values, W1, b1, W2)
    nc = _get_nc()
    res = run_bass_kernel_spmd(nc, in_maps, core_ids=list(range(NCORES)))

    context = np.concatenate(
        [res.results[c]["context"] for c in range(NCORES)], axis=0
    ).reshape(B, 1, H)
    alphas = np.concatenate(
        [res.results[c]["alphas"] for c in range(NCORES)], axis=0
    ).reshape(B, S, 1)
    return (context.astype(np.float32), alphas.astype(np.float32))


# revision 16
# speedup vs baseline: 1.0920x; 1.0920x over previous
"""Additive-attention kernel for Trainium2, data-parallel over 8 NeuronCores.

Reference computation (B=16, S=2048, H=1024):
    concat = [broadcast(q), keys]                 # [B,S,2H]
    h      = relu(concat @ W1 + b1)               # [B,S,H]
    scores = h @ W2 + b2                          # [B,S,1]
    alphas = softmax(scores, axis=S)
    context = alphas^T @ values                   # [B,1,H]
    returns (context, alphas)

Kernel strategy (per core, 2 batches):
  - concat @ W1 == q @ W1[:H] + keys @ W1[H:]  -> the q part is computed once
    per batch ([B,H]) in fp32r, folded with b1 into a per-partition bias; the
    big matmul is keys @ W1k only (half the naive FLOPs).
  - The keys/values datapath runs in fp32r (full-rate fp32 matmul mode,
    rounding applied by the gpsimd DMA-cast in the DMA datapath):
    hT[h',s] = relu(W1k^T @ keys^T + qpre) with keys tiles PE-transposed on
    chip (measured: bf16 matmuls are no faster than fp32r on this toolchain,
    so fp32r wins on precision at equal speed).
  - scores come out as rows [1,512]; exp on ACT with accum_out giving the
    partial softmax denominators for free. b2 is dropped entirely (softmax is
    shift-invariant) and so is the max-subtraction (scores are provably small
    for this module, so exp cannot overflow in fp32).
  - context = (sum_s exp(s) * values[s]) * (1/Z), accumulated in PSUM (fp32)
    across the whole sequence, normalized once at the end.
"""

import sys

for _p in ("/opt/trn_rl_repo",):
    if _p not in sys.path:
        sys.path.append(_p)

import numpy as np

B, S, H = 16, 2048, 1024
NCORES = 8
BPC = B // NCORES          # batches per core
P = 128                    # partitions
FO = H // P                # 8 f-chunks (contraction dim of W1k)
MO = H // P                # 8 h'-chunks
SBLK = 512                 # s-block (matmul moving free dim)
NST = S // SBLK            # 4 s-blocks per batch
NSO = SBLK // P            # 4 s-subchunks per block

_NC_CACHE = {}


def _build_nc():
    import concourse.bass as bass  # noqa: F401
    import concourse.mybir as mybir
    import concourse.tile as tile
    from concourse import bacc
    from concourse.bass import ts, ds
    from concourse.masks import make_identity

    dt = mybir.dt
    f32, f32r, bf16 = dt.float32, dt.float32r, dt.bfloat16
    AF = mybir.ActivationFunctionType

    nc = bacc.Bacc()

    keys_d = nc.dram_tensor("keys", [BPC, S, H], f32, kind="ExternalInput")
    values_d = nc.dram_tensor("values", [BPC, S, H], f32, kind="ExternalInput")
    # host-prepped layouts: [P, FO, ...] with the contraction dim on partitions
    w1k_d = nc.dram_tensor("w1k", [P, FO, H], f32, kind="ExternalInput")
    w1q_d = nc.dram_tensor("w1q", [P, FO, H], f32, kind="ExternalInput")
    qT_d = nc.dram_tensor("qT", [P, FO, BPC], f32, kind="ExternalInput")
    b1T_d = nc.dram_tensor("b1T", [P, MO], f32, kind="ExternalInput")
    w2T_d = nc.dram_tensor("w2T", [P, MO], f32, kind="ExternalInput")

    ctx_d = nc.dram_tensor("context", [BPC, H], f32, kind="ExternalOutput")
    alphas_d = nc.dram_tensor("alphas", [BPC, S], f32, kind="ExternalOutput")

    with tile.TileContext(nc) as tc:
        with (
            tc.tile_pool(name="const", bufs=1) as const_pool,
            tc.tile_pool(name="w1qp", bufs=2) as w1q_pool,
            tc.tile_pool(name="knat", bufs=3) as knat_pool,
            tc.tile_pool(name="vnat", bufs=3) as vnat_pool,
            tc.tile_pool(name="ktsb", bufs=2) as kt_pool,
            tc.tile_pool(name="htsb", bufs=2) as ht_pool,
            tc.tile_pool(name="alph", bufs=1) as alph_pool,
            tc.tile_pool(name="small", bufs=2) as small_pool,
            tc.tile_pool(name="ktps", bufs=2, space="PSUM") as ktp_pool,
            tc.tile_pool(name="hps", bufs=2, space="PSUM") as h_psum_pool,
            tc.tile_pool(name="scps", bufs=1, space="PSUM") as sc_pool,
            tc.tile_pool(name="etps", bufs=1, space="PSUM") as etp_pool,
            tc.tile_pool(name="ctxps", bufs=2, space="PSUM") as ctx_pool,
        ):
            # ---- prefetch the first keys/values blocks (gpsimd queue head,
            # casting fp32 -> bf16 in the DMA datapath) ----
            kv_tiles = {}

            def issue_kv(b, st):
                knat = []
                for hb in range(2):
                    kt_ = knat_pool.tile(
                        [P, 2, H], f32r, tag="knat", name=f"knat_{b}_{st}_{hb}"
                    )
                    nc.gpsimd.dma_start(
                        kt_[:],
                        keys_d[b, ds(st * SBLK + hb * 2 * P, 2 * P), :].rearrange(
                            "(so si) f -> si so f", si=P
                        ),
                    )
                    knat.append(kt_)
                vnat = []
                for hb in range(2):
                    vt = vnat_pool.tile(
                        [P, 2, H], f32r, tag="vnat", name=f"vnat_{b}_{st}_{hb}"
                    )
                    nc.gpsimd.dma_start(
                        vt[:],
                        values_d[b, ds(st * SBLK + hb * 2 * P, 2 * P), :].rearrange(
                            "(so si) f -> si so f", si=P
                        ),
                    )
                    vnat.append(vt)
                return knat, vnat

            kv_tiles[(0, 0)] = issue_kv(0, 0)

            # ---- constants / weights ----
            iden = const_pool.tile([P, P], f32)
            make_identity(nc, iden)
            iden_r = const_pool.tile([P, P], f32r)
            nc.vector.tensor_copy(out=iden_r[:], in_=iden[:])

            qT_sb = const_pool.tile([P, FO, BPC], f32r)
            nc.gpsimd.dma_start(qT_sb[:], qT_d[:])
            w2T_sb = const_pool.tile([P, MO], f32r)
            nc.gpsimd.dma_start(w2T_sb[:], w2T_d[:])
            b1T_sb = const_pool.tile([P, MO], f32)
            nc.sync.dma_start(b1T_sb[:], b1T_d[:])

            kv_tiles[(0, 1)] = issue_kv(0, 1)

            # W1 rides the sync (HWDGE) queue in fp32 chunks so it does not
            # serialize behind the keys/values stream; DVE rounds the chunks
            # (W1q to f32r for the small query matmul, W1k to bf16).
            w1k_sb = const_pool.tile([P, FO, H], f32r)
            qpreT_sb = const_pool.tile([P, MO, BPC], f32)
            qrow_ps = [
                ctx_pool.tile([BPC, SBLK], f32, tag="ctx", name=f"qrow{n}")
                for n in range(2)
            ]
            for fo in range(FO):
                w1q_raw = w1q_pool.tile([P, H], f32, tag="wraw", name=f"w1qraw{fo}")
                nc.sync.dma_start(w1q_raw[:], w1q_d[:, fo, :])
                w1q_sb = w1q_pool.tile([P, H], f32r, tag="w1q")
                nc.vector.tensor_copy(out=w1q_sb[:], in_=w1q_raw[:])
                # qrow[b, h'] = qT.T @ W1q  (queries are the 2-col stationary)
                for n in range(2):
                    nc.tensor.matmul(
                        qrow_ps[n][:],
                        lhsT=qT_sb[:, fo, :],
                        rhs=w1q_sb[:, ts(n, SBLK)],
                        start=(fo == 0),
                        stop=(fo == FO - 1),
                    )
            for fo in range(FO):
                wraw = w1q_pool.tile([P, H], f32, tag="kraw", name=f"w1kraw{fo}")
                nc.scalar.dma_start(wraw[:], w1k_d[:, fo, :])
                nc.vector.tensor_copy(out=w1k_sb[:, fo, :], in_=wraw[:])

            # qpreT[h', b] = qrow^T + b1 broadcast (PE-transposed in P-chunks)
            qrow_sb = const_pool.tile([BPC, H], f32)
            for n in range(2):
                nc.vector.tensor_copy(
                    out=qrow_sb[:, ts(n, SBLK)], in_=qrow_ps[n][:]
                )
            for mo in range(MO):
                qtp = etp_pool.tile([P, NSO], f32, tag="etp", name=f"qtp{mo}")
                nc.tensor.transpose(
                    qtp[:, :BPC],
                    qrow_sb[:, ts(mo, P)],
                    iden[:BPC, :BPC],
                )
                nc.vector.tensor_tensor(
                    qpreT_sb[:, mo, :],
                    qtp[:, :BPC],
                    b1T_sb[:, mo : mo + 1].to_broadcast((P, BPC)),
                    mybir.AluOpType.add,
                )

            # ---- main loop ----
            for b in range(BPC):
                ctx_ps = [
                    ctx_pool.tile([1, SBLK], f32, tag="ctx", name=f"ctx_{b}_{n}")
                    for n in range(2)
                ]
                alph = alph_pool.tile([1, S], f32)
                zpart = small_pool.tile([1, NST], f32, tag="zpart")

                for st in range(NST):
                    knat, vnat = kv_tiles.pop((b, st), None) or issue_kv(b, st)
                    nxt = (b, st + 1) if st + 1 < NST else (b + 1, 0)
                    if nxt[0] < BPC and nxt not in kv_tiles:
                        kv_tiles[nxt] = issue_kv(*nxt)

                    # keys^T tiles: [f-chunk on partitions, s]
                    kT = kt_pool.tile([P, FO, SBLK], f32r)
                    for fo in range(FO):
                        ktp = ktp_pool.tile([P, SBLK], f32r, tag="ktp")
                        for so in range(NSO):
                            nc.tensor.transpose(
                                ktp[:, ts(so, P)],
                                knat[so // 2][:, so % 2, ts(fo, P)],
                                iden_r[:],
                            )
                        nc.vector.tensor_copy(
                            out=kT[:, fo, :], in_=ktp[:].bitcast(f32)
                        )

                    # hT[h'-chunk, s] = relu(W1k^T @ keys^T + qpre)
                    hT = ht_pool.tile([P, MO, SBLK], f32r)
                    for mo in range(MO):
                        hps = h_psum_pool.tile([P, SBLK], f32, tag="hps")
                        for fo in range(FO):
                            nc.tensor.matmul(
                                hps,
                                lhsT=w1k_sb[:, fo, ts(mo, P)],
                                rhs=kT[:, fo, :],
                                start=(fo == 0),
                                stop=(fo == FO - 1),
                            )
                        nc.scalar.activation(
                            out=hT[:, mo, :],
                            in_=hps,
                            func=AF.Relu,
                            bias=qpreT_sb[:, mo, b : b + 1],
                            scale=1.0,
                        )

                    # scores row [1, SBLK] = w2^T @ hT
                    scp = sc_pool.tile([1, SBLK], f32, tag="scp")
                    for mo in range(MO):
                        nc.tensor.matmul(
                            scp,
                            lhsT=w2T_sb[:, mo : mo + 1],
                            rhs=hT[:, mo, :],
                            start=(mo == 0),
                            stop=(mo == MO - 1),
                        )

                    # exp + partial softmax denominator in one ACT op
                    nc.scalar.activation(
                        out=alph[:, ts(st, SBLK)],
                        in_=scp,
                        func=AF.Exp,
                        accum_out=zpart[:, st : st + 1],
                    )

                    # transpose exp row -> [s on partitions, 1] for context
                    etp = etp_pool.tile([P, NSO], f32, tag="etp")
                    for so in range(NSO):
                        nc.tensor.transpose(
                            etp[:, so : so + 1],
                            alph[:, ds(st * SBLK + so * P, P)],
                            iden[0:1, 0:1],
                        )
                    expT = small_pool.tile([P, NSO], f32r, tag="expT")
                    nc.vector.tensor_copy(out=expT[:], in_=etp[:])

                    # context accumulation: ctx[n] += expT^T @ values
                    for n in range(2):
                        for so in range(NSO):
                            nc.tensor.matmul(
                                ctx_ps[n],
                                lhsT=expT[:, so : so + 1],
                                rhs=vnat[so // 2][:, so % 2, ts(n, SBLK)],
                                start=(st == 0 and so == 0),
                                stop=(st == NST - 1 and so == NSO - 1),
                            )

                # ---- batch epilogue: normalize ----
                zsum = small_pool.tile([1, 1], f32, tag="zsum")
                nc.vector.reduce_sum(zsum, zpart, axis=mybir.AxisListType.X)
                rinv = small_pool.tile([1, 1], f32, tag="rinv")
                nc.vector.reciprocal(rinv, zsum)

                nc.vector.tensor_scalar_mul(alph[:], alph[:], rinv)
                nc.sync.dma_start(alphas_d[b : b + 1, :], alph[:])

                ctx_sb = alph_pool.tile([1, H], f32, tag="ctxsb")
                for n in range(2):
                    nc.vector.tensor_scalar_mul(
                        ctx_sb[:, ts(n, SBLK)], ctx_ps[n][:], rinv
                    )
                nc.sync.dma_start(ctx_d[b : b + 1, :], ctx_sb[:])

    nc.finalize()
    return nc


def _get_nc():
    if "nc" not in _NC_CACHE:
        _NC_CACHE["nc"] = _build_nc()
    return _NC_CACHE["nc"]


def make_in_maps(queries, keys, values, W1, b1, W2):
    w1k_h = np.ascontiguousarray(W1[H:].reshape(FO, P, H).transpose(1, 0, 2))
    w1q_h = np.ascontiguousarray(W1[:H].reshape(FO, P, H).transpose(1, 0, 2))
    b1T_h = np.ascontiguousarray(b1.reshape(MO, P).T)
    w2T_h = np.ascontiguousarray(W2[:, 0].reshape(MO, P).T)

    in_maps = []
    for c in range(NCORES):
        bsl = slice(c * BPC, (c + 1) * BPC)
        qT_h = np.ascontiguousarray(
            queries[bsl].T.reshape(FO, P, BPC).transpose(1, 0, 2)
        )
        in_maps.append(
            {
                "keys": np.ascontiguousarray(keys[bsl]),
                "values": np.ascontiguousarray(values[bsl]),
                "w1k": w1k_h,
                "w1q": w1q_h,
                "qT": qT_h,
                "b1T": b1T_h,
                "w2T": w2T_h,
            }
        )
    return in_maps


def kernel(queries, keys, values, W1, b1, W2, b2, **_ignored):
    from concourse.bass_utils import run_bass_kernel_spmd

    queries = np.asarray(queries, dtype=np.float32)
    keys = np.asarray(keys, dtype=np.float32)
    values = np.asarray(values, dtype=np.float32)
    W1 = np.asarray(W1, dtype=np.float32)
    b1 = np.asarray(b1, dtype=np.float32)
    W2 = np.asarray(W2, dtype=np.float32)

    in_maps = make_in_maps(queries, keys, values, W1, b1, W2)
    nc = _get_nc()
    res = run_bass_kernel_spmd(nc, in_maps, core_ids=list(range(NCORES)))

    context = np.concatenate(
        [res.results[c]["context"] for c in range(NCORES)], axis=0
    ).reshape(B, 1, H)
    alphas = np.concatenate(
        [res.results[c]["alphas"] for c in range(NCORES)], axis=0
    ).reshape(B, S, 1)
    return (context.astype(np.float32), alphas.astype(np.float32))


# revision 18
# speedup vs baseline: 1.1555x; 1.0582x over previous
"""Additive-attention kernel for Trainium2, data-parallel over 8 NeuronCores.

Reference computation (B=16, S=2048, H=1024):
    concat = [broadcast(q), keys]                 # [B,S,2H]
    h      = relu(concat @ W1 + b1)               # [B,S,H]
    scores = h @ W2 + b2                          # [B,S,1]
    alphas = softmax(scores, axis=S)
    context = alphas^T @ values                   # [B,1,H]
    returns (context, alphas)

Kernel strategy (per core, 2 batches):
  - concat @ W1 == q @ W1[:H] + keys @ W1[H:]  -> the q part is computed once
    per batch ([B,H]) in fp32r, folded with b1 into a per-partition bias; the
    big matmul is keys @ W1k only (half the naive FLOPs).
  - The keys/values datapath runs in fp32r (full-rate fp32 matmul mode,
    rounding applied by the gpsimd DMA-cast in the DMA datapath):
    hT[h',s] = relu(W1k^T @ keys^T + qpre) with keys tiles PE-transposed on
    chip (measured: bf16 matmuls are no faster than fp32r on this toolchain,
    so fp32r wins on precision at equal speed).
  - scores come out as rows [1,512]; exp on ACT with accum_out giving the
    partial softmax denominators for free. b2 is dropped entirely (softmax is
    shift-invariant) and so is the max-subtraction (scores are provably small
    for this module, so exp cannot overflow in fp32).
  - context = (sum_s exp(s) * values[s]) * (1/Z), accumulated in PSUM (fp32)
    across the whole sequence, normalized once at the end.
"""

import sys

for _p in ("/opt/trn_rl_repo",):
    if _p not in sys.path:
        sys.path.append(_p)

import numpy as np

B, S, H = 16, 2048, 1024
NCORES = 8
BPC = B // NCORES          # batches per core
P = 128                    # partitions
FO = H // P                # 8 f-chunks (contraction dim of W1k)
MO = H // P                # 8 h'-chunks
SBLK = 512                 # s-block (matmul moving free dim)
NST = S // SBLK            # 4 s-blocks per batch
NSO = SBLK // P            # 4 s-subchunks per block

_NC_CACHE = {}


def _build_nc():
    import concourse.bass as bass  # noqa: F401
    import concourse.mybir as mybir
    import concourse.tile as tile
    from concourse import bacc
    from concourse.bass import ts, ds
    from concourse.masks import make_identity

    dt = mybir.dt
    f32, f32r, bf16 = dt.float32, dt.float32r, dt.bfloat16
    AF = mybir.ActivationFunctionType

    nc = bacc.Bacc()

    keys_d = nc.dram_tensor("keys", [BPC, S, H], f32, kind="ExternalInput")
    values_d = nc.dram_tensor("values", [BPC, S, H], f32, kind="ExternalInput")
    # host-prepped layouts: [P, FO, ...] with the contraction dim on partitions
    w1k_d = nc.dram_tensor("w1k", [P, FO, H], f32, kind="ExternalInput")
    w1q_d = nc.dram_tensor("w1q", [MO // 2, P, FO, 2 * P], f32, kind="ExternalInput")
    qT_d = nc.dram_tensor("qT", [P, FO, BPC], f32, kind="ExternalInput")
    b1T_d = nc.dram_tensor("b1T", [P, MO], f32, kind="ExternalInput")
    w2T_d = nc.dram_tensor("w2T", [P, MO], f32, kind="ExternalInput")

    ctx_d = nc.dram_tensor("context", [BPC, H], f32, kind="ExternalOutput")
    alphas_d = nc.dram_tensor("alphas", [BPC, S], f32, kind="ExternalOutput")

    with tile.TileContext(nc) as tc:
        with (
            tc.tile_pool(name="const", bufs=1) as const_pool,
            tc.tile_pool(name="w1qp", bufs=2) as w1q_pool,
            tc.tile_pool(name="knat", bufs=4) as knat_pool,
            tc.tile_pool(name="vnat", bufs=2) as vnat_pool,
            tc.tile_pool(name="ktsb", bufs=2) as kt_pool,
            tc.tile_pool(name="htsb", bufs=2) as ht_pool,
            tc.tile_pool(name="alph", bufs=1) as alph_pool,
            tc.tile_pool(name="small", bufs=2) as small_pool,
            tc.tile_pool(name="ktps", bufs=2, space="PSUM") as ktp_pool,
            tc.tile_pool(name="hps", bufs=2, space="PSUM") as h_psum_pool,
            tc.tile_pool(name="scps", bufs=1, space="PSUM") as sc_pool,
            tc.tile_pool(name="etps", bufs=1, space="PSUM") as etp_pool,
            tc.tile_pool(name="ctxps", bufs=2, space="PSUM") as ctx_pool,
        ):
            # ---- keys/values streaming (gpsimd SWDGE queue, fp32 -> fp32r
            # cast in the DMA datapath). Keys are prefetched ahead; values are
            # issued per-block since they are consumed late (context matmul).
            k_tiles = {}

            def issue_k(b, st):
                knat = []
                for hb in range(2):
                    kt_ = knat_pool.tile(
                        [P, 2, H], f32r, tag="knat", name=f"knat_{b}_{st}_{hb}"
                    )
                    nc.gpsimd.dma_start(
                        kt_[:],
                        keys_d[b, ds(st * SBLK + hb * 2 * P, 2 * P), :].rearrange(
                            "(so si) f -> si so f", si=P
                        ),
                    )
                    knat.append(kt_)
                return knat

            def issue_v(b, st):
                vnat = []
                for hb in range(2):
                    vt = vnat_pool.tile(
                        [P, 2, H], f32r, tag="vnat", name=f"vnat_{b}_{st}_{hb}"
                    )
                    nc.gpsimd.dma_start(
                        vt[:],
                        values_d[b, ds(st * SBLK + hb * 2 * P, 2 * P), :].rearrange(
                            "(so si) f -> si so f", si=P
                        ),
                    )
                    vnat.append(vt)
                return vnat

            k_tiles[(0, 0)] = issue_k(0, 0)

            # ---- constants / weights ----
            iden = const_pool.tile([P, P], f32)
            make_identity(nc, iden)
            iden_r = const_pool.tile([P, P], f32r)
            nc.vector.tensor_copy(out=iden_r[:], in_=iden[:])

            qT_sb = const_pool.tile([P, FO, BPC], f32r)
            nc.gpsimd.dma_start(qT_sb[:], qT_d[:])
            w2T_sb = const_pool.tile([P, MO], f32r)
            nc.gpsimd.dma_start(w2T_sb[:], w2T_d[:])
            b1T_sb = const_pool.tile([P, MO], f32)
            nc.sync.dma_start(b1T_sb[:], b1T_d[:])

            k_tiles[(0, 1)] = issue_k(0, 1)

            # W1 rides the sync (HWDGE) queue in fp32 chunks so it does not
            # serialize behind the keys/values stream: first W1k (needed by the
            # first h-matmul group), then W1q in 4 pair-chunks ordered by
            # output column so qpreT becomes available 2 mo-chunks at a time
            # (the first relu only waits for the first pair).
            w1k_sb = const_pool.tile([P, FO, H], f32r)
            for fo in range(FO):
                wraw = w1q_pool.tile([P, H], f32, tag="kraw", name=f"w1kraw{fo}")
                nc.sync.dma_start(wraw[:], w1k_d[:, fo, :])
                nc.vector.tensor_copy(out=w1k_sb[:, fo, :], in_=wraw[:])

            qpreT_sb = const_pool.tile([P, MO, BPC], f32)
            NPAIR = 2 * P  # 256 columns per W1q chunk (f32r full rate needs >=256)
            for pr in range(MO // 2):
                w1q_raw = w1q_pool.tile(
                    [P, FO, NPAIR], f32, tag="wraw", name=f"w1qraw{pr}"
                )
                nc.sync.dma_start(w1q_raw[:], w1q_d[pr])
                w1q_sb = w1q_pool.tile([P, FO, NPAIR], f32r, tag="w1q")
                nc.vector.tensor_copy(out=w1q_sb[:], in_=w1q_raw[:])
                # qrow[b, h'pair] = qT.T @ W1q_pair (queries = 2-col stationary)
                qrow_ps = ctx_pool.tile(
                    [BPC, NPAIR], f32, tag="ctx", name=f"qrow{pr}"
                )
                for fo in range(FO):
                    nc.tensor.matmul(
                        qrow_ps[:],
                        lhsT=qT_sb[:, fo, :],
                        rhs=w1q_sb[:, fo, :],
                        start=(fo == 0),
                        stop=(fo == FO - 1),
                    )
                qrow_sb = small_pool.tile(
                    [BPC, NPAIR], f32, tag="qrow", name=f"qrow_sb{pr}"
                )
                nc.vector.tensor_copy(out=qrow_sb[:], in_=qrow_ps[:])
                for mi in range(2):
                    mo = 2 * pr + mi
                    qtp = etp_pool.tile([P, NSO], f32, tag="etp", name=f"qtp{mo}")
                    nc.tensor.transpose(
                        qtp[:, :BPC],
                        qrow_sb[:, ts(mi, P)],
                        iden[:BPC, :BPC],
                    )
                    nc.vector.tensor_tensor(
                        qpreT_sb[:, mo, :],
                        qtp[:, :BPC],
                        b1T_sb[:, mo : mo + 1].to_broadcast((P, BPC)),
                        mybir.AluOpType.add,
                    )

            # ---- main loop ----
            for b in range(BPC):
                ctx_ps = [
                    ctx_pool.tile([1, SBLK], f32, tag="ctx", name=f"ctx_{b}_{n}")
                    for n in range(2)
                ]
                alph = alph_pool.tile([1, S], f32)
                zpart = small_pool.tile([1, NST], f32, tag="zpart")

                for st in range(NST):
                    knat = k_tiles.pop((b, st), None) or issue_k(b, st)
                    vnat = issue_v(b, st)
                    nxt = (b, st + 1) if st + 1 < NST else (b + 1, 0)
                    if nxt[0] < BPC and nxt not in k_tiles:
                        k_tiles[nxt] = issue_k(*nxt)

                    # keys^T tiles: [f-chunk on partitions, s]
                    kT = kt_pool.tile([P, FO, SBLK], f32r)
                    for fo in range(FO):
                        ktp = ktp_pool.tile([P, SBLK], f32r, tag="ktp")
                        for so in range(NSO):
                            nc.tensor.transpose(
                                ktp[:, ts(so, P)],
                                knat[so // 2][:, so % 2, ts(fo, P)],
                                iden_r[:],
                            )
                        nc.vector.tensor_copy(
                            out=kT[:, fo, :], in_=ktp[:].bitcast(f32)
                        )

                    # hT[h'-chunk, s] = relu(W1k^T @ keys^T + qpre)
                    hT = ht_pool.tile([P, MO, SBLK], f32r)
                    for mo in range(MO):
                        hps = h_psum_pool.tile([P, SBLK], f32, tag="hps")
                        for fo in range(FO):
                            nc.tensor.matmul(
                                hps,
                                lhsT=w1k_sb[:, fo, ts(mo, P)],
                                rhs=kT[:, fo, :],
                                start=(fo == 0),
                                stop=(fo == FO - 1),
                            )
                        nc.scalar.activation(
                            out=hT[:, mo, :],
                            in_=hps,
                            func=AF.Relu,
                            bias=qpreT_sb[:, mo, b : b + 1],
                            scale=1.0,
                        )

                    # scores row [1, SBLK] = w2^T @ hT
                    scp = sc_pool.tile([1, SBLK], f32, tag="scp")
                    for mo in range(MO):
                        nc.tensor.matmul(
                            scp,
                            lhsT=w2T_sb[:, mo : mo + 1],
                            rhs=hT[:, mo, :],
                            start=(mo == 0),
                            stop=(mo == MO - 1),
                        )

                    # exp + partial softmax denominator in one ACT op
                    nc.scalar.activation(
                        out=alph[:, ts(st, SBLK)],
                        in_=scp,
                        func=AF.Exp,
                        accum_out=zpart[:, st : st + 1],
                    )

                    # transpose exp row -> [s on partitions, 1] for context
                    etp = etp_pool.tile([P, NSO], f32, tag="etp")
                    for so in range(NSO):
                        nc.tensor.transpose(
                            etp[:, so : so + 1],
                            alph[:, ds(st * SBLK + so * P, P)],
                            iden[0:1, 0:1],
                        )
                    expT = small_pool.tile([P, NSO], f32r, tag="expT")
                    nc.vector.tensor_copy(out=expT[:], in_=etp[:])

                    # context accumulation: ctx[n] += expT^T @ values
                    for n in range(2):
                        for so in range(NSO):
                            nc.tensor.matmul(
                                ctx_ps[n],
                                lhsT=expT[:, so : so + 1],
                                rhs=vnat[so // 2][:, so % 2, ts(n, SBLK)],
                                start=(st == 0 and so == 0),
                                stop=(st == NST - 1 and so == NSO - 1),
                            )

                # ---- batch epilogue: normalize ----
                zsum = small_pool.tile([1, 1], f32, tag="zsum")
                nc.vector.reduce_sum(zsum, zpart, axis=mybir.AxisListType.X)
                rinv = small_pool.tile([1, 1], f32, tag="rinv")
                nc.vector.reciprocal(rinv, zsum)

                nc.vector.tensor_scalar_mul(alph[:], alph[:], rinv)
                nc.sync.dma_start(alphas_d[b : b + 1, :], alph[:])

                ctx_sb = alph_pool.tile([1, H], f32, tag="ctxsb")
                for n in range(2):
                    nc.vector.tensor_scalar_mul(
                        ctx_sb[:, ts(n, SBLK)], ctx_ps[n][:], rinv
                    )
                nc.sync.dma_start(ctx_d[b : b + 1, :], ctx_sb[:])

    nc.finalize()
    return nc


def _get_nc():
    if "nc" not in _NC_CACHE:
        _NC_CACHE["nc"] = _build_nc()
    return _NC_CACHE["nc"]


def make_in_maps(queries, keys, values, W1, b1, W2):
    w1k_h = np.ascontiguousarray(W1[H:].reshape(FO, P, H).transpose(1, 0, 2))
    w1q_h = np.ascontiguousarray(
        W1[:H].reshape(FO, P, MO // 2, 2 * P).transpose(2, 1, 0, 3)
    )
    b1T_h = np.ascontiguousarray(b1.reshape(MO, P).T)
    w2T_h = np.ascontiguousarray(W2[:, 0].reshape(MO, P).T)

    in_maps = []
    for c in range(NCORES):
        bsl = slice(c * BPC, (c + 1) * BPC)
        qT_h = np.ascontiguousarray(
            queries[bsl].T.reshape(FO, P, BPC).transpose(1, 0, 2)
        )
        in_maps.append(
            {
                "keys": np.ascontiguousarray(keys[bsl]),
                "values": np.ascontiguousarray(values[bsl]),
                "w1k": w1k_h,
                "w1q": w1q_h,
                "qT": qT_h,
                "b1T": b1T_h,
                "w2T": w2T_h,
            }
        )
    return in_maps


def kernel(queries, keys, values, W1, b1, W2, b2, **_ignored):
    from concourse.bass_utils import run_bass_kernel_spmd

    queries = np.asarray(queries, dtype=np.float32)
    keys = np.asarray(keys, dtype=np.float32)
    values = np.asarray(values, dtype=np.float32)
    W1 = np.asarray(W1, dtype=np.float32)
    b1 = np.asarray(b1, dtype=np.float32)
    W2 = np.asarray(W2, dtype=np.float32)

    in_maps = make_in_maps(queries, keys, values, W1, b1, W2)
    nc = _get_nc()
    res = run_bass_kernel_spmd(nc, in_maps, core_ids=list(range(NCORES)))

    context = np.concatenate(
        [res.results[c]["context"] for c in range(NCORES)], axis=0
    ).reshape(B, 1, H)
    alphas = np.concatenate(
        [res.results[c]["alphas"] for c in range(NCORES)], axis=0
    ).reshape(B, S, 1)
    return (context.astype(np.float32), alphas.astype(np.float32))
